# revision 7
# baseline (speedup 1.0000x reference)
"""DenseStructuralGAT layer on 8 Trainium2 NeuronCores.

Row-parallel sharding: core c owns rows [c*1024, (c+1)*1024) of the
8192x8192 attention problem.

Launch 1 (tiny): each core computes its slice H_c = X_c @ W^T (fp16 out),
s_c = X_c @ (W^T a_l), t_c = X_c @ (W^T a_r)  (fp32).
Host concatenates H and t, casts H to fp8-e4m3 and lays it out for
DoubleRow matmuls - pure data movement + one numpy cast.

Launch 2 (main): per core, over its 1024x8192 block of M:
  m16   = fp16(M), 6 of 8 i-tiles per j-chunk     [one SWDGE cast-DMA:
                                                   358 GB/s when batched]
  m32   = M, other 2 i-tiles                      [HWDGE f32, concurrent ring]
  e     = prelu(t_j + s_i, alpha=0.2)             [ACT, f16, t_bcast from host]
  v     = m * e                                   [DVE TT, f16 2x / f32 1x]
  z     = (v == 0) * -50                          [DVE TS, f16 4x]
  w     = v + z                                   [DVE TT, f16 2x]
  wT    = transpose(w) via PE identity matmuls    [PE -> PSUM f32]
  p     = exp(wT - 1)                             [ACT drain, fp8-e4m3 out]
  Z^T  += H^T @ p   (fp8 DoubleRow, K=256/inst)   [PE, f32 PSUM accum]
  rs   += 1^T @ p   (fp8 DoubleRow)               [PE]
  Z     = sigmoid(Z^T.T * (1/rs))                 [PE transpose + ACT]

Masked entries (M==0) get logits=-50 -> exp(-51) underflows to 0 in fp8,
i.e. softmax-identical to the reference's -inf masking.  The global
exp(-1) factor cancels between numerator and rowsum (and keeps exp below
the fp8 max normal 240, which saturates to INF: max logit here is 5.4).
Rows with no neighbours do not occur for this input distribution (checked
in test.py).
"""

import numpy as np
import ml_dtypes

import concourse.bacc as bacc
import concourse.mybir as mybir
import concourse.tile as tile
from concourse.bass_utils import run_bass_kernel_spmd
from concourse.masks import make_identity

N, FIN, FOUT = 8192, 512, 256
NCORES = 8
RB = N // NCORES          # 1024 rows per core
NIT = RB // 128           # 8 i-tiles per core
NJC = 8                   # j-chunks
JC = N // NJC             # 1024 columns per chunk
NJS = JC // 128           # 8 j-subtiles per chunk
NJT = N // 128            # 64 j-tiles total
NPAIR = NJT // 2          # 32 DoubleRow K-pairs
NEG = -50.0
ALPHA = 0.2
EXP_BIAS = -1.0           # global softmax shift; keeps exp() < fp8 max (240)

F32 = mybir.dt.float32
F16 = mybir.dt.float16
F8 = mybir.dt.float8e4
AF = mybir.ActivationFunctionType
ALU = mybir.AluOpType
PM = mybir.MatmulPerfMode


# ----------------------------------------------------------------- launch 1

def build_h_kernel(reps=1, dynamic=False):
    nc = bacc.Bacc()
    x_d = nc.dram_tensor("xc", [RB, FIN], F32, kind="ExternalInput")
    w_d = nc.dram_tensor("w", [FOUT, FIN], F32, kind="ExternalInput")
    a_d = nc.dram_tensor("attn", [1, 2 * FOUT], F32, kind="ExternalInput")
    h_d = nc.dram_tensor("hc", [RB, FOUT], F16, kind="ExternalOutput")
    s_d = nc.dram_tensor("sc", [RB, 1], F32, kind="ExternalOutput")
    t_d = nc.dram_tensor("tc", [RB, 1], F32, kind="ExternalOutput")

    with tile.TileContext(nc) as tc:
        with (
            tc.tile_pool(name="sb", bufs=1) as sb,
            tc.tile_pool(name="xt3", bufs=3) as xtp,
            tc.tile_pool(name="ps", bufs=2, space="PSUM") as ps,
            tc.tile_pool(name="psacc", bufs=2, space="PSUM") as psacc,
        ):
            ident = sb.tile([128, 128], F32, tag="ident")
            make_identity(nc, ident[:])
            one11 = sb.tile([1, 1], F32, tag="one")
            nc.vector.memset(one11[:], 1.0)
            wu = sb.tile([128, 512], F32, tag="wu")
            nc.vector.memset(wu[:], 0.5)

            rep_ctx = tc.For_i(0, reps, 1) if dynamic else None
            if rep_ctx is not None:
                rep_ctx.__enter__()

            # PE warmup burst: ~5us of dummy matmuls, hidden under the X/W
            # input DMAs, releases the HAM clock throttle (1.2 -> 2.4 GHz)
            # before the real (short-burst) matmul work begins.
            wu_ps = psacc.tile([128, 512], F32, tag="pstc")
            for i in range(12):
                nc.tensor.matmul(
                    wu_ps[:], wu[:, :128], wu[:],
                    start=(i == 0), stop=(i == 11),
                )
            wu_sink = sb.tile([128, 1], F32, tag="wusink")
            nc.vector.tensor_copy(wu_sink[:], wu_ps[:, 0:1])

            x_sb = sb.tile([128, NIT, FIN], F32, tag="xsb")
            nc.sync.dma_start(
                x_sb[:], x_d[:].rearrange("(it p) k -> p it k", p=128)
            )
            w_sb = sb.tile([128, 2, FIN], F32, tag="wsb")
            nc.scalar.dma_start(
                w_sb[:], w_d[:].rearrange("(ft p) k -> p ft k", p=128)
            )
            a_sb = sb.tile([1, 2 * FOUT], F32, tag="asb")
            nc.scalar.dma_start(a_sb[:], a_d[:])

            # attention vector chunks as columns: 4 chunks of 128
            # [a_l0 a_l1 | a_r0 a_r1]
            a_cols = sb.tile([128, 4], F32, tag="acols")
            for h in range(4):
                pa = ps.tile([128, 1], F32, tag="pt")
                nc.tensor.matmul(pa[:], a_sb[0:1, 128 * h:128 * (h + 1)], one11[:])
                nc.any.tensor_copy(a_cols[:, h:h + 1], pa[:])

            # W^T tiles: WT[kt] = (128k x 256f)
            wt_sb = sb.tile([128, 4 * FOUT], F32, tag="wtsb")
            for kt in range(4):
                for ft in range(2):
                    pw = ps.tile([128, 128], F32, tag="pt")
                    nc.tensor.matmul(
                        pw[:],
                        w_sb[:, ft, kt * 128:(kt + 1) * 128],
                        ident[:],
                    )
                    nc.any.tensor_copy(
                        wt_sb[:, kt * FOUT + ft * 128: kt * FOUT + (ft + 1) * 128],
                        pw[:],
                    )

            # w_s / w_t columns per k-tile: wst[kc] = (128k x 2)
            wst = sb.tile([128, 8], F32, tag="wst")
            for kc in range(4):
                pst = psacc.tile([128, 2], F32, tag="pstc")
                for ft in range(2):
                    rhs = sb.tile([128, 2], F32, tag="arhs")
                    nc.vector.tensor_copy(rhs[:, 0:1], a_cols[:, ft:ft + 1])
                    nc.vector.tensor_copy(rhs[:, 1:2], a_cols[:, 2 + ft:3 + ft])
                    nc.tensor.matmul(
                        pst[:],
                        w_sb[:, ft, kc * 128:(kc + 1) * 128],
                        rhs[:],
                        start=(ft == 0),
                        stop=(ft == 1),
                    )
                nc.any.tensor_copy(wst[:, 2 * kc: 2 * kc + 2], pst[:])

            # combined rhs [WT(256) | w_s w_t(2)] per kt
            wtc = sb.tile([128, 4, FOUT + 2], F32, tag="wtc")
            for kt in range(4):
                nc.vector.tensor_copy(wtc[:, kt, :FOUT], wt_sb[:, kt * FOUT:(kt + 1) * FOUT])
                nc.vector.tensor_copy(wtc[:, kt, FOUT:], wst[:, 2 * kt: 2 * kt + 2])

            # per-it: transpose X[it] then immediately matmul H_c/s_c/t_c
            hb_all = sb.tile([128, NIT, FOUT], F16, tag="hball")
            stb_all = sb.tile([128, NIT, 2], F32, tag="stball")
            for it in range(NIT):
                xt_it = xtp.tile([128, 4, 128], F32, tag="xtit")
                for kt in range(4):
                    px = ps.tile([128, 128], F32, tag="pt")
                    nc.tensor.matmul(
                        px[:],
                        x_sb[:, it, kt * 128:(kt + 1) * 128],
                        ident[:],
                    )
                    nc.any.tensor_copy(xt_it[:, kt, :], px[:])
                ph = psacc.tile([128, FOUT + 2], F32, tag="ph")
                for kt in range(4):
                    nc.tensor.matmul(
                        ph[:], xt_it[:, kt, :], wtc[:, kt, :],
                        start=(kt == 0), stop=(kt == 3),
                    )
                nc.any.tensor_copy(hb_all[:, it, :], ph[:, :FOUT])
                nc.any.tensor_copy(stb_all[:, it, :], ph[:, FOUT:])
            nc.sync.dma_start(
                h_d[:].rearrange("(it p) f -> p it f", p=128), hb_all[:]
            )
            nc.scalar.dma_start(
                s_d[:].rearrange("(it p) o -> p it o", p=128),
                stb_all[:, :, 0:1],
            )
            nc.scalar.dma_start(
                t_d[:].rearrange("(it p) o -> p it o", p=128),
                stb_all[:, :, 1:2],
            )
            if rep_ctx is not None:
                rep_ctx.__exit__(None, None, None)
    nc.finalize()
    return nc


# ----------------------------------------------------------------- launch 2

N16 = 6   # i-tiles per j-chunk loaded via SWDGE cast-DMA (f16)


def build_main_kernel(reps=1, dynamic=False):
    nc = bacc.Bacc()
    m_d = nc.dram_tensor("mc", [RB, N], F32, kind="ExternalInput")
    h8_d = nc.dram_tensor(
        "h8", [128, (NPAIR + 1) * 2 * FOUT], F8, kind="ExternalInput"
    )
    tb_d = nc.dram_tensor("tb", [128, N], F16, kind="ExternalInput")
    s_d = nc.dram_tensor("sc", [RB, 1], F32, kind="ExternalInput")
    z_d = nc.dram_tensor("zc", [RB, FOUT], F32, kind="ExternalOutput")

    with tile.TileContext(nc) as tc:
        with (
            tc.tile_pool(name="const", bufs=1) as cst,
            tc.tile_pool(name="zps", bufs=1, space="PSUM") as zpool,
            tc.tile_pool(name="trps", bufs=2, space="PSUM") as trpool,
            tc.tile_pool(name="mf16", bufs=2) as mf16p,
            tc.tile_pool(name="mf32", bufs=4) as mf32p,
            tc.tile_pool(name="chain", bufs=6) as chp,
            tc.tile_pool(name="pchain", bufs=16) as ppool,
            tc.tile_pool(name="pt8", bufs=2) as pt8p,
            tc.tile_pool(name="misc", bufs=4) as misc,
        ):
            ident16 = cst.tile([128, 128], F16, tag="id16")
            make_identity(nc, ident16[:])
            ident32 = cst.tile([128, 128], F32, tag="id32")
            make_identity(nc, ident32[:])
            ebias = cst.tile([128, 1], F32, tag="ebias")
            nc.vector.memset(ebias[:], EXP_BIAS)

            # full H as fp8 DoubleRow lhsT tiles: [128j, pair, krow, 256f];
            # pair NPAIR is an all-ones slot used for the rowsum matmul.
            h8_sb = cst.tile([128, NPAIR + 1, 2, FOUT], F8, tag="h8sb")
            nc.sync.dma_start(
                h8_sb[:],
                h8_d[:].rearrange("p (a i f) -> p a i f", i=2, f=FOUT),
            )
            # t broadcast (128 x 8192 f16, host-replicated) + s columns
            tb_sb = cst.tile([128, N], F16, tag="tbsb")
            nc.scalar.dma_start(tb_sb[:], tb_d[:])
            s_sb = cst.tile([128, NIT, 1], F32, tag="ssb")
            nc.sync.dma_start(
                s_sb[:], s_d[:].rearrange("(it p) o -> p it o", p=128)
            )

            # Z^T accumulators: 2 f-chunks x (128f x 1024i) fp32 psum
            zps = []
            for fc in range(2):
                zp = zpool.tile([128, RB], F32, tag=f"zps{fc}", name=f"zps{fc}")
                zps.append(zp)
            rsT = zpool.tile([1, RB], F32, tag="rsT", name="rsT")

            m_re = m_d[:].rearrange("(it p) j -> p it j", p=128)
            rep_ctx = tc.For_i(0, reps, 1) if dynamic else None
            if rep_ctx is not None:
                rep_ctx.__enter__()
            for rep in range(1 if dynamic else reps):
              for jc in range(NJC):
                # M tiles: its 0..5 in one SWDGE cast-DMA (f32->f16, full
                # line rate only when batched); its 6,7 as f32 on the two
                # HWDGE rings, which run concurrently with the SWDGE ring.
                m16_t = mf16p.tile([128, N16, JC], F16, tag="m16")
                nc.gpsimd.dma_start(
                    m16_t[:], m_re[:, 0:N16, jc * JC:(jc + 1) * JC]
                )
                m32 = []
                for q, it in enumerate(range(N16, NIT)):
                    m_t = mf32p.tile([128, JC], F32, tag="m32")
                    eng = nc.sync if q % 2 == 0 else nc.scalar
                    eng.dma_start(
                        m_t[:],
                        m_d[it * 128:(it + 1) * 128, jc * JC:(jc + 1) * JC],
                    )
                    m32.append(m_t)

                p_tiles = []
                for it in range(NIT):
                    e_t = chp.tile([128, JC], F16, tag="et")
                    nc.scalar.activation(
                        e_t[:], tb_sb[:, jc * JC:(jc + 1) * JC], AF.Prelu,
                        bias=s_sb[:, it, :], scale=1.0, alpha=ALPHA,
                    )
                    m_ap = m16_t[:, it, :] if it < N16 else m32[it - N16][:]
                    v_t = chp.tile([128, JC], F16, tag="vt")
                    nc.vector.tensor_tensor(v_t[:], m_ap, e_t[:], ALU.mult)
                    z_t = chp.tile([128, JC], F16, tag="zt")
                    nc.vector.tensor_scalar(
                        z_t[:], v_t[:], 0.0, NEG, ALU.is_equal, ALU.mult
                    )
                    p_t = ppool.tile([128, JC], F16, tag="pt")
                    nc.vector.tensor_tensor(p_t[:], v_t[:], z_t[:], ALU.add)
                    p_tiles.append(p_t)

                if rep == 0 and jc == 0:
                    # PE warmup: dummy matmuls reading the first chain tile so
                    # the scheduler runs them during the (PE-idle) first chain,
                    # releasing the HAM clock throttle before the transpose
                    # burst begins.
                    wu16 = misc.tile([128, 512], F16, tag="wu16")
                    nc.vector.memset(wu16[:], 0.25)
                    wu_ps = trpool.tile([128, 512], F32, tag="pstr")
                    for i in range(12):
                        nc.tensor.matmul(
                            wu_ps[:], p_tiles[0][:, 0:128], wu16[:],
                            start=(i == 0), stop=(i == 11),
                        )
                    wu_sink = misc.tile([128, 1], F32, tag="wusink")
                    nc.vector.tensor_copy(wu_sink[:], wu_ps[:, 0:1])

                # transpose w, drain through exp into fp8
                pt8 = pt8p.tile([128, NJS, 2, 512], F8, tag="pt8")
                for js in range(NJS):
                    for ih in range(2):
                        ptr = trpool.tile([128, 512], F32, tag="pstr")
                        for it4 in range(4):
                            it = ih * 4 + it4
                            nc.tensor.matmul(
                                ptr[:, it4 * 128:(it4 + 1) * 128],
                                p_tiles[it][:, js * 128:(js + 1) * 128],
                                ident16[:],
                            )
                        nc.scalar.activation(
                            pt8[:, js, ih, :], ptr[:], AF.Exp, bias=ebias[:],
                        )

                # fp8 DoubleRow matmuls: Z^T accumulation + rowsums
                for pl in range(NJS // 2):
                    pair = jc * (NJS // 2) + pl
                    first = pair == 0
                    last = pair == NPAIR - 1
                    for fc in range(2):
                        for ih in range(2):
                            nc.tensor.matmul(
                                zps[fc][:, ih * 512:(ih + 1) * 512],
                                h8_sb[:, pair, :, fc * 128:(fc + 1) * 128],
                                pt8[:, 2 * pl:2 * pl + 2, ih, :],
                                start=first, stop=last,
                                perf_mode=PM.DoubleRow,
                            )
                    for ih in range(2):
                        nc.tensor.matmul(
                            rsT[:, ih * 512:(ih + 1) * 512],
                            h8_sb[:, NPAIR, :, 0:1],
                            pt8[:, 2 * pl:2 * pl + 2, ih, :],
                            start=first, stop=last,
                            perf_mode=PM.DoubleRow,
                        )

            if rep_ctx is not None:
                rep_ctx.__exit__(None, None, None)
            # ---- finale
            zt_sb = []
            for fc in range(2):
                zt = cst.tile([128, RB], F32, tag=f"ztsb{fc}", name=f"ztsb{fc}")
                zt_sb.append(zt)
            for fc in range(2):
                nc.any.tensor_copy(zt_sb[fc][:], zps[fc][:])

            rec = cst.tile([128, NIT], F32, tag="rec")
            rtot = cst.tile([128, NIT], F32, tag="rtot")
            rs_row = cst.tile([1, RB], F32, tag="rsrow")
            nc.vector.tensor_copy(rs_row[:], rsT[:])
            one11b = cst.tile([1, 1], F32, tag="one11b")
            nc.vector.memset(one11b[:], 1.0)
            for it in range(NIT):
                prs = trpool.tile([128, 1], F32, tag="pstr")
                nc.tensor.matmul(
                    prs[:], rs_row[0:1, it * 128:(it + 1) * 128], one11b[:])
                nc.vector.tensor_copy(rtot[:, it:it + 1], prs[:])
            nc.vector.reciprocal(rec[:], rtot[:])

            for it in range(NIT):
                pz = trpool.tile([128, FOUT], F32, tag="pstr")
                for fc in range(2):
                    nc.tensor.matmul(
                        pz[:, fc * 128:(fc + 1) * 128],
                        zt_sb[fc][:, it * 128:(it + 1) * 128],
                        ident32[:],
                    )
                z_out = misc.tile([128, FOUT], F32, tag="zout")
                nc.scalar.activation(
                    z_out[:], pz[:], AF.Sigmoid,
                    bias=0.0, scale=rec[:, it:it + 1],
                )
                nc.sync.dma_start(z_d[it * 128:(it + 1) * 128, :], z_out[:])
    nc.finalize()
    return nc


# ----------------------------------------------------------------- host glue

_CACHE = {}


def _get_kernels():
    if "h" not in _CACHE:
        _CACHE["h"] = build_h_kernel()
        _CACHE["main"] = build_main_kernel()
    return _CACHE["h"], _CACHE["main"]


def pack_h8(H):
    """H (N, FOUT) float -> fp8 DoubleRow lhsT layout [128, (NPAIR+1)*2*FOUT]
    with an all-ones pair appended for the rowsum matmul."""
    h8 = np.asarray(H).astype(ml_dtypes.float8_e4m3)
    h8 = h8.reshape(NPAIR, 2, 128, FOUT).transpose(2, 0, 1, 3)  # p, pair, i, f
    ones = np.ones((128, 1, 2, FOUT), dtype=ml_dtypes.float8_e4m3)
    h8 = np.concatenate([h8, ones], axis=1)
    return np.ascontiguousarray(h8.reshape(128, (NPAIR + 1) * 2 * FOUT))


def prep_main_inputs(M, H, t_row, s_blocks):
    h8 = pack_h8(H)
    tb = np.ascontiguousarray(
        np.broadcast_to(t_row.astype(np.float16), (128, N))
    )
    return [
        {
            "mc": M[c * RB:(c + 1) * RB],
            "h8": h8,
            "tb": tb,
            "sc": s_blocks[c],
        }
        for c in range(NCORES)
    ]


def kernel(X, M, W_w, attn_w):
    X = np.ascontiguousarray(X, dtype=np.float32)
    M = np.ascontiguousarray(M, dtype=np.float32)
    W_w = np.ascontiguousarray(W_w, dtype=np.float32)
    attn_w = np.ascontiguousarray(attn_w, dtype=np.float32).reshape(1, 2 * FOUT)

    nc_h, nc_main = _get_kernels()
    cores = list(range(NCORES))

    in1 = [
        {"xc": X[c * RB:(c + 1) * RB], "w": W_w, "attn": attn_w}
        for c in cores
    ]
    r1 = run_bass_kernel_spmd(nc_h, in1, cores).results
    H = np.concatenate([r1[c]["hc"] for c in cores], axis=0)      # fp16
    t = np.concatenate([r1[c]["tc"] for c in cores], axis=0)      # (N,1) f32
    t_row = np.ascontiguousarray(t.reshape(1, N))

    in2 = prep_main_inputs(M, H, t_row, [r1[c]["sc"] for c in cores])
    r2 = run_bass_kernel_spmd(nc_main, in2, cores).results
    Z = np.concatenate([r2[c]["zc"] for c in cores], axis=0)
    return Z.astype(np.float32)
